# revision 1
# baseline (speedup 1.0000x reference)
# Mistral-style GQA attention layer (QKV proj + RoPE + causal attention +
# o_proj), tensor-parallel over heads across 8 NeuronCores.
#
# Sharding (8-way TP over heads): core c owns q heads [4c..4c+4) and kv head c.
#   - w_qkv rows sharded: 4 q-head blocks + 1 k block + 1 v block per core
#   - w_o columns sharded: each core computes a partial o_proj output,
#     host sums the 8 partials (the "all-reduce").
#
# Device kernel (identical SPMD program, per-core data):
#   phase 1: qkvT = Wc @ X^T (outputs TRANSPOSED: [dim, t]) + inline RoPE on
#            q/k rows, spilled to DRAM scratch.
#   phase 2: per (batch, head): S^T = K Q^T on PE, exp on ACT (no max-sub:
#            scores are O(5) and fp32 exp is safe), causal mask via
#            multiplicative 0/1 tiles on DVE, softmax denominator via
#            ones-matmul over the partition (key) axis, P@V with V as the
#            stationary operand (needs V natural layout -> 128x128 PE
#            transposes of V^T), normalize at the end.
#   phase 3: o_partial = attn @ Wo_c^T using attn^T tiles as lhsT.
#
# All matmuls run as float32r (FP22 multiply, fp32 accumulate): full PE rate
# at moving-dim >= 256, ~1e-4 relative error.

import os
from contextlib import nullcontext
from dataclasses import dataclass

import numpy as np

import concourse.bass as bass
from concourse import bacc
import concourse.mybir as mybir
import concourse.tile as tile
from concourse.bass import ds, ts
from concourse.bass_utils import run_bass_kernel_spmd
from concourse.masks import make_identity

F32 = mybir.dt.float32
F32R = mybir.dt.float32r
EXPF = mybir.ActivationFunctionType.Exp
F16 = mybir.dt.float16


@dataclass(frozen=True)
class Cfg:
    T: int = 4096          # total tokens (B*S)
    H: int = 4096          # hidden size
    S: int = 1024          # seq len
    nq: int = 4            # q heads per core
    n_cores: int = 8
    D: int = 128           # head dim
    mm16: bool = False     # fp16 matmul paths instead of fp32r

    @property
    def B(self):
        return self.T // self.S

    @property
    def KO(self):  # contraction tiles for qkv proj
        return self.H // 128

    @property
    def NM(self):  # qkv output row-tiles per core (q heads + k + v)
        return self.nq + 2

    @property
    def QW(self):  # q tile width in attention
        return min(512, self.S)

    @property
    def NJ(self):
        return self.S // self.QW

    @property
    def NKT(self):  # key tiles per batch
        return self.S // 128

    @property
    def NSLAB(self):
        return self.T // 256


FULL = Cfg()


def r32(ap):
    return ap.bitcast(F32R)


def build_nc(cfg: Cfg, loop: int | None = None, phases=(1, 2, 3)) -> bass.Bass:  # returns compiled Bacc
    nc = bacc.Bacc("TRN2", target_bir_lowering=False, debug=False, num_devices=cfg.n_cores)
    MDT = F16 if cfg.mm16 else F32R          # matmul-path storage dtype
    TDT = F16 if cfg.mm16 else F32           # table dtype (cos/sin/masks)
    T, H, S, nq, D = cfg.T, cfg.H, cfg.S, cfg.nq, cfg.D
    KO, NM, QW, NJ, NKT, B = cfg.KO, cfg.NM, cfg.QW, cfg.NJ, cfg.NKT, cfg.B
    NRT = QW // 128  # number of diagonal mask offsets
    scale = 1.0 / np.sqrt(D)

    xT = nc.dram_tensor("xT", [KO, 128, T], MDT, kind="ExternalInput")
    wqkvT = nc.dram_tensor("wqkvT", [KO, 128, NM * 128], MDT, kind="ExternalInput")
    woT = nc.dram_tensor("woT", [nq, 128, H], MDT, kind="ExternalInput")
    cosT = nc.dram_tensor("cosT", [128, S], TDT, kind="ExternalInput")
    sinT = nc.dram_tensor("sinT", [128, S], TDT, kind="ExternalInput")
    masksT = nc.dram_tensor("masksT", [128, NRT, QW], TDT, kind="ExternalInput")
    identT = nc.dram_tensor("identT", [128, 128], MDT, kind="ExternalInput")
    onesT = nc.dram_tensor("onesT", [128, 1], MDT, kind="ExternalInput")
    out = nc.dram_tensor("o_partial", [T, H], F16, kind="ExternalOutput")

    with tile.TileContext(nc) as tc:
        with (
            tc.tile_pool(name="psum", bufs=8, space="PSUM") as psum,
            tc.tile_pool(name="consts", bufs=1) as consts,
            tc.tile_pool(name="dram", bufs=1, space="DRAM") as dram,
        ):
            qkv_sp = dram.tile([NM, 128, T], MDT)

            ident = consts.tile([128, 128], MDT)
            nc.sync.dma_start(ident, identT[:])
            ones = consts.tile([128, 1], MDT)
            nc.sync.dma_start(ones, onesT[:])
            masks_sb = consts.tile([128, NRT, QW], TDT)
            nc.sync.dma_start(masks_sb, masksT[:])
            nbias = consts.tile([128, 1], F32)
            nc.gpsimd.memset(nbias, -4.0)

            with (tc.For_i(0, loop, 1) if loop else nullcontext()):
                if 1 in phases:
                    # ---------------- phase 1: QKV projection + RoPE ----------------
                    with (
                        tc.tile_pool(name="wq", bufs=1) as wq_pool,
                        tc.tile_pool(name="xin", bufs=2) as xin,
                        tc.tile_pool(name="stage", bufs=2) as stage,
                        tc.tile_pool(name="rot", bufs=2) as rot_pool,
                        tc.tile_pool(name="tab", bufs=1) as tab,
                    ):
                        w_all = wq_pool.tile([128, KO, NM * 128], MDT)
                        nc.sync.dma_start(w_all, wqkvT[:].rearrange("k p m -> p k m"))
                        cos_sb = tab.tile([128, S], TDT)
                        nc.sync.dma_start(cos_sb, cosT[:])
                        sin_sb = tab.tile([128, S], TDT)
                        nc.sync.dma_start(sin_sb, sinT[:])

                        SLAB = min(512, cfg.T)
                        KH = min(8, KO)  # k-tiles per x-slab chunk
                        NCH = KO // KH
                        RH = min(256, SLAB)  # RoPE column-chunk
                        for n in range(cfg.T // SLAB):
                            tsl = ds(n * SLAB, SLAB)
                            # PSUM tiles first so matmuls can start per-chunk
                            pss = [
                                psum.tile([128, SLAB], F32, tag="bank", name=f"qk_ps{m}")
                                for m in range(NM)
                            ]
                            for ch in range(NCH):
                                xt = xin.tile([128, KH, SLAB], MDT, tag="xh")
                                nc.sync.dma_start(
                                    xt, xT[ds(ch * KH, KH), :, tsl].rearrange("k p t -> p k t")
                                )
                                for m in range(NM):
                                    for k in range(KH):
                                        nc.tensor.matmul(
                                            pss[m],
                                            w_all[:, ch * KH + k, ts(m, 128)],
                                            xt[:, k, :],
                                            start=(ch == 0 and k == 0),
                                            stop=(ch == NCH - 1 and k == KH - 1),
                                        )
                            st = stage.tile([128, NM, SLAB], MDT)
                            for m in range(NM):
                                nc.scalar.copy(st[:, m, :], pss[m])
                            # RoPE on q heads + k head (rows 0..nq), not v
                            for rh in range(SLAB // RH):
                                rsl = ds(rh * RH, RH)
                                rot = rot_pool.tile([128, nq + 1, RH], MDT, tag="rot")
                                nc.sync.dma_start(rot[0:64], st[64:128, 0 : nq + 1, rsl])
                                nc.sync.dma_start(rot[64:128], st[0:64, 0 : nq + 1, rsl])
                                s0 = (n * SLAB + rh * RH) % S
                                cos_b = cos_sb[:, None, ds(s0, RH)].to_broadcast(
                                    (128, nq + 1, RH)
                                )
                                sin_b = sin_sb[:, None, ds(s0, RH)].to_broadcast(
                                    (128, nq + 1, RH)
                                )
                                nc.vector.tensor_mul(
                                    st[:, 0 : nq + 1, rsl], st[:, 0 : nq + 1, rsl], cos_b
                                )
                                nc.vector.tensor_mul(rot, rot, sin_b)
                                nc.vector.tensor_add(
                                    st[:, 0 : nq + 1, rsl], st[:, 0 : nq + 1, rsl], rot
                                )
                            nc.sync.dma_start(
                                qkv_sp[:, :, tsl].rearrange("m p t -> p m t"), st
                            )

                if 2 in phases:
                    # ---------------- phase 2: attention ----------------
                    with (
                        tc.tile_pool(name="attn", bufs=1) as attn_pool,
                        tc.tile_pool(name="kv", bufs=2) as kv_pool,
                        tc.tile_pool(name="vn", bufs=2) as vn_pool,
                        tc.tile_pool(name="qh", bufs=3) as q_pool,
                        tc.tile_pool(name="pt", bufs=12) as pt_pool,
                        tc.tile_pool(name="sm", bufs=4) as sm_pool,
                    ):
                        attnT = attn_pool.tile([128, B * nq, S], MDT)
                        for b in range(B):
                            bsl = ds(b * S, S)
                            kT = kv_pool.tile([128, S], MDT, tag="kT")
                            nc.sync.dma_start(kT, qkv_sp[nq, :, bsl])
                            vT = kv_pool.tile([128, S], MDT, tag="vT")
                            nc.sync.dma_start(vT, qkv_sp[nq + 1, :, bsl])
                            vn = vn_pool.tile([128, NKT, 128], MDT)
                            for kt in range(NKT):
                                tp = psum.tile([128, 128], MDT, tag="bank")
                                nc.tensor.transpose(tp, vT[:, ts(kt, 128)], ident)
                                nc.vector.tensor_copy(vn[:, kt, :], tp)
                            for h in range(nq):
                                q = q_pool.tile([128, S], MDT)
                                nc.sync.dma_start(q, qkv_sp[h, :, bsl])
                                for j in range(NJ):
                                    nkt = (j + 1) * NRT
                                    qsl = ds(j * QW, QW)
                                    pts = []
                                    for kt in range(nkt):
                                        sp = psum.tile([128, QW], F32, tag="bank")
                                        nc.tensor.matmul(
                                            sp,
                                            kT[:, ts(kt, 128)],
                                            q[:, qsl],
                                            start=True,
                                            stop=True,
                                        )
                                        pt = pt_pool.tile([128, QW], MDT, tag="pt")
                                        nc.scalar.activation(pt, sp, EXPF, bias=nbias[:, 0:1], scale=scale)
                                        if kt >= j * NRT:
                                            nc.vector.tensor_mul(
                                                pt, pt, masks_sb[:, kt - j * NRT, :]
                                            )
                                        pts.append(pt)
                                    sum_ps = psum.tile([1, QW], F32, tag="bank")
                                    for i, pt in enumerate(pts):
                                        nc.tensor.matmul(
                                            sum_ps,
                                            ones,
                                            pt,
                                            start=(i == 0),
                                            stop=(i == len(pts) - 1),
                                        )
                                    recip = sm_pool.tile([1, QW], F32, tag="recip")
                                    nc.vector.reciprocal(recip, sum_ps)
                                    rb = sm_pool.tile([128, QW], F32, tag="rb")
                                    nc.gpsimd.partition_broadcast(rb, recip)
                                    pv = psum.tile([128, QW], F32, tag="bank")
                                    for i, pt in enumerate(pts):
                                        nc.tensor.matmul(
                                            pv,
                                            vn[:, i, :],
                                            pt,
                                            start=(i == 0),
                                            stop=(i == len(pts) - 1),
                                        )
                                    nc.vector.tensor_mul(
                                        attnT[:, b * nq + h, ds(j * QW, QW)], pv, rb
                                    )

                        if 3 in phases:
                            # ---------------- phase 3: o_proj ----------------
                            with (
                                tc.tile_pool(name="wo", bufs=2) as wo_pool,
                                tc.tile_pool(name="ot", bufs=4) as ot_pool,
                            ):
                                for nh in range(H // 512):
                                    hsl = ds(nh * 512, 512)
                                    wo_t = wo_pool.tile([128, nq, 512], MDT)
                                    nc.sync.dma_start(
                                        wo_t, woT[:, :, hsl].rearrange("a p n -> p a n")
                                    )
                                    for tm in range(T // 128):
                                        b = (tm * 128) // S
                                        tloc = tm * 128 - b * S
                                        ps = psum.tile([128, 512], F32, tag="bank")
                                        for a in range(nq):
                                            nc.tensor.matmul(
                                                ps,
                                                attnT[:, b * nq + a, ds(tloc, 128)],
                                                wo_t[:, a, :],
                                                start=(a == 0),
                                                stop=(a == nq - 1),
                                            )
                                        ot = ot_pool.tile([128, 512], F16)
                                        if tm % 2 == 0:
                                            nc.scalar.copy(ot, ps)
                                        else:
                                            nc.vector.tensor_copy(ot, ps)
                                        nc.sync.dma_start(out[ts(tm, 128), hsl], ot)
    nc.compile()
    return nc


def prep_core_inputs(cfg: Cfg, hidden, w_qkv, w_o, core: int):
    """Build the per-core input map (C-contiguous, matmul-path dtype)."""
    ndt = np.float16 if cfg.mm16 else np.float32
    T, H, S, nq, D = cfg.T, cfg.H, cfg.S, cfg.nq, cfg.D
    NQ_TOT = cfg.n_cores * nq
    xT = np.ascontiguousarray(hidden.T.astype(ndt)).reshape(cfg.KO, 128, T)
    q0 = core * nq * D
    rows = list(range(q0, q0 + nq * D))
    rows += list(range(NQ_TOT * D + core * D, NQ_TOT * D + (core + 1) * D))
    kv_heads = cfg.n_cores  # one kv head per core
    rows += list(
        range((NQ_TOT + kv_heads) * D + core * D, (NQ_TOT + kv_heads) * D + (core + 1) * D)
    )
    wqkvT = np.ascontiguousarray(w_qkv[rows, :].T.astype(ndt)).reshape(cfg.KO, 128, cfg.NM * 128)
    woT = np.ascontiguousarray(
        w_o[:, core * nq * D : (core + 1) * nq * D].T.astype(ndt)
    ).reshape(nq, 128, H)
    return {"xT": xT, "wqkvT": wqkvT, "woT": woT}


def prep_shared_inputs(cfg: Cfg, rope_theta=10000.0):
    ndt = np.float16 if cfg.mm16 else np.float32
    S, D = cfg.S, cfg.D
    NRT = cfg.QW // 128
    inv = 1.0 / (rope_theta ** (np.arange(0, D, 2, dtype=np.float64) / D))
    ang = np.arange(S, dtype=np.float64)[:, None] * inv[None, :]  # [S, 64]
    cos = np.cos(ang).T.astype(np.float32)  # [64, S]
    sin = np.sin(ang).T.astype(np.float32)
    cosT = np.concatenate([cos, cos], axis=0)  # [128, S]
    sinT = np.concatenate([-sin, sin], axis=0)
    kl = np.arange(128)[:, None, None]
    r = np.arange(NRT)[None, :, None]
    ql = np.arange(cfg.QW)[None, None, :]
    masksT = ((r * 128 + kl) <= ql).astype(np.float32)
    return {
        "cosT": np.ascontiguousarray(cosT.astype(ndt)),
        "sinT": np.ascontiguousarray(sinT.astype(ndt)),
        "masksT": np.ascontiguousarray(masksT.astype(ndt)),
        "identT": np.eye(128, dtype=ndt),
        "onesT": np.ones((128, 1), dtype=ndt),
    }


_CACHE = {}
LAST_EXEC_NS = None


def _get_nc(cfg: Cfg) -> bass.Bass:
    if cfg not in _CACHE:
        _CACHE[cfg] = build_nc(cfg)
    return _CACHE[cfg]


def kernel(hidden_states=None, w_qkv=None, w_o=None, seq_len=None, **_):
    cfg = FULL
    hidden = np.asarray(hidden_states, dtype=np.float32)
    w_qkv = np.asarray(w_qkv, dtype=np.float32)
    w_o = np.asarray(w_o, dtype=np.float32)

    nc = _get_nc(cfg)
    shared = prep_shared_inputs(cfg)
    in_maps = []
    for c in range(cfg.n_cores):
        m = dict(shared)
        m.update(prep_core_inputs(cfg, hidden, w_qkv, w_o, c))
        in_maps.append(m)

    trace = os.environ.get("KERNEL_TRACE", "0") == "1"
    res = run_bass_kernel_spmd(
        nc, in_maps, core_ids=list(range(cfg.n_cores)), trace=trace
    )
    global LAST_EXEC_NS
    if res.exec_time_ns is not None:
        LAST_EXEC_NS = res.exec_time_ns
        print(f"HW exec time: {res.exec_time_ns} ns")
        if res.instructions_and_trace is not None:
            print(f"trace: {res.instructions_and_trace[1]}")
    outs = [r["o_partial"] for r in res.results]
    acc = np.zeros(outs[0].shape, dtype=np.float32)
    for o in outs:
        acc += o.astype(np.float32)
    return acc



# revision 3
# speedup vs baseline: 1.3266x; 1.3266x over previous
# Mistral-style GQA attention layer (QKV proj + RoPE + causal attention +
# o_proj), tensor-parallel over heads across 8 NeuronCores.
#
# Sharding (8-way TP over heads): core c owns q heads [4c..4c+4) and kv head c.
#   - w_qkv rows sharded: 4 q-head blocks + 1 k block + 1 v block per core
#   - w_o columns sharded: each core computes a partial o_proj output,
#     host sums the 8 partials (the "all-reduce").
#
# Device kernel (identical SPMD program, per-core data):
#   phase 1: qkvT = Wc @ X^T (outputs TRANSPOSED: [dim, t]) + inline RoPE on
#            q/k rows, spilled to DRAM scratch.
#   phase 2: per (batch, head): S^T = K Q^T on PE, exp on ACT (no max-sub:
#            scores are O(5) and fp32 exp is safe), causal mask via
#            multiplicative 0/1 tiles on DVE, softmax denominator via
#            ones-matmul over the partition (key) axis, P@V with V as the
#            stationary operand (needs V natural layout -> 128x128 PE
#            transposes of V^T), normalize at the end.
#   phase 3: o_partial = attn @ Wo_c^T using attn^T tiles as lhsT.
#
# All matmuls run as float32r (FP22 multiply, fp32 accumulate): full PE rate
# at moving-dim >= 256, ~1e-4 relative error.

import os
from contextlib import nullcontext
from dataclasses import dataclass

import numpy as np

import concourse.bass as bass
from concourse import bacc
import concourse.mybir as mybir
import concourse.tile as tile
from concourse.bass import ds, ts
from concourse.bass_utils import run_bass_kernel_spmd
from concourse.masks import make_identity

F32 = mybir.dt.float32
F32R = mybir.dt.float32r
EXPF = mybir.ActivationFunctionType.Exp
F16 = mybir.dt.float16


@dataclass(frozen=True)
class Cfg:
    T: int = 4096          # total tokens (B*S)
    H: int = 4096          # hidden size
    S: int = 1024          # seq len
    nq: int = 4            # q heads per core
    n_cores: int = 8
    D: int = 128           # head dim
    mm16: bool = False     # fp16 matmul paths instead of fp32r

    @property
    def B(self):
        return self.T // self.S

    @property
    def KO(self):  # contraction tiles for qkv proj
        return self.H // 128

    @property
    def NM(self):  # qkv output row-tiles per core (q heads + k + v)
        return self.nq + 2

    @property
    def QW(self):  # q tile width in attention
        return min(512, self.S)

    @property
    def NJ(self):
        return self.S // self.QW

    @property
    def NKT(self):  # key tiles per batch
        return self.S // 128

    @property
    def NSLAB(self):
        return self.T // 256


FULL = Cfg()


def r32(ap):
    return ap.bitcast(F32R)


def build_nc(cfg: Cfg, loop: int | None = None, phases=(1, 2, 3)) -> bass.Bass:  # returns compiled Bacc
    nc = bacc.Bacc("TRN2", target_bir_lowering=False, debug=False, num_devices=cfg.n_cores)
    MDT = F16 if cfg.mm16 else F32R          # matmul-path storage dtype
    TDT = F16 if cfg.mm16 else F32           # table dtype (cos/sin/masks)
    T, H, S, nq, D = cfg.T, cfg.H, cfg.S, cfg.nq, cfg.D
    KO, NM, QW, NJ, NKT, B = cfg.KO, cfg.NM, cfg.QW, cfg.NJ, cfg.NKT, cfg.B
    NRT = QW // 128  # number of diagonal mask offsets
    scale = 1.0 / np.sqrt(D)

    xT = nc.dram_tensor("xT", [KO, 128, T], MDT, kind="ExternalInput")
    wqkvT = nc.dram_tensor("wqkvT", [KO, 128, NM * 128], MDT, kind="ExternalInput")
    woT = nc.dram_tensor("woT", [nq, 128, H], MDT, kind="ExternalInput")
    cosT = nc.dram_tensor("cosT", [128, S], TDT, kind="ExternalInput")
    sinT = nc.dram_tensor("sinT", [128, S], TDT, kind="ExternalInput")
    masksT = nc.dram_tensor("masksT", [128, NRT, QW], TDT, kind="ExternalInput")
    identT = nc.dram_tensor("identT", [128, 128], MDT, kind="ExternalInput")
    onesT = nc.dram_tensor("onesT", [128, 1], MDT, kind="ExternalInput")
    out = nc.dram_tensor("o_partial", [T, H], F16, kind="ExternalOutput")

    with tile.TileContext(nc) as tc:
        with (
            tc.tile_pool(name="psum", bufs=8, space="PSUM") as psum,
            tc.tile_pool(name="consts", bufs=1) as consts,
            tc.tile_pool(name="dram", bufs=1, space="DRAM") as dram,
        ):
            qkv_sp = dram.tile([NM, 128, T], MDT)

            ident = consts.tile([128, 128], MDT)
            nc.sync.dma_start(ident, identT[:])
            ones = consts.tile([128, 1], MDT)
            nc.sync.dma_start(ones, onesT[:])
            masks_sb = consts.tile([128, NRT, QW], TDT)
            nc.sync.dma_start(masks_sb, masksT[:])
            nbias = consts.tile([128, 1], F32)
            nc.gpsimd.memset(nbias, -4.0)

            with (tc.For_i(0, loop, 1) if loop else nullcontext()):
                if 1 in phases:
                    # ---------------- phase 1: QKV projection + RoPE ----------------
                    with (
                        tc.tile_pool(name="wq", bufs=1) as wq_pool,
                        tc.tile_pool(name="xin", bufs=2) as xin,
                        tc.tile_pool(name="stage", bufs=2) as stage,
                        tc.tile_pool(name="rot", bufs=2) as rot_pool,
                        tc.tile_pool(name="tab", bufs=1) as tab,
                    ):
                        w_all = wq_pool.tile([128, KO, NM * 128], MDT)
                        nc.sync.dma_start(w_all, wqkvT[:].rearrange("k p m -> p k m"))
                        cos_sb = tab.tile([128, S], TDT)
                        nc.sync.dma_start(cos_sb, cosT[:])
                        sin_sb = tab.tile([128, S], TDT)
                        nc.sync.dma_start(sin_sb, sinT[:])

                        SLAB = min(512, cfg.T)
                        KH = min(8, KO)  # k-tiles per x-slab chunk
                        NCH = KO // KH
                        RH = min(256, SLAB)  # RoPE column-chunk
                        for n in range(cfg.T // SLAB):
                            tsl = ds(n * SLAB, SLAB)
                            # PSUM tiles first so matmuls can start per-chunk
                            pss = [
                                psum.tile([128, SLAB], F32, tag="bank", name=f"qk_ps{m}")
                                for m in range(NM)
                            ]
                            for ch in range(NCH):
                                xt = xin.tile([128, KH, SLAB], MDT, tag="xh")
                                nc.sync.dma_start(
                                    xt, xT[ds(ch * KH, KH), :, tsl].rearrange("k p t -> p k t")
                                )
                                for m in range(NM):
                                    for k in range(KH):
                                        nc.tensor.matmul(
                                            pss[m],
                                            w_all[:, ch * KH + k, ts(m, 128)],
                                            xt[:, k, :],
                                            start=(ch == 0 and k == 0),
                                            stop=(ch == NCH - 1 and k == KH - 1),
                                        )
                            st = stage.tile([128, NM, SLAB], MDT)
                            for m in range(NM):
                                nc.scalar.copy(st[:, m, :], pss[m])
                            # RoPE on q heads + k head (rows 0..nq), not v
                            for rh in range(SLAB // RH):
                                rsl = ds(rh * RH, RH)
                                rot = rot_pool.tile([128, nq + 1, RH], MDT, tag="rot")
                                nc.sync.dma_start(rot[0:64], st[64:128, 0 : nq + 1, rsl])
                                nc.sync.dma_start(rot[64:128], st[0:64, 0 : nq + 1, rsl])
                                s0 = (n * SLAB + rh * RH) % S
                                cos_b = cos_sb[:, None, ds(s0, RH)].to_broadcast(
                                    (128, nq + 1, RH)
                                )
                                sin_b = sin_sb[:, None, ds(s0, RH)].to_broadcast(
                                    (128, nq + 1, RH)
                                )
                                nc.vector.tensor_mul(
                                    st[:, 0 : nq + 1, rsl], st[:, 0 : nq + 1, rsl], cos_b
                                )
                                nc.vector.tensor_mul(rot, rot, sin_b)
                                nc.vector.tensor_add(
                                    st[:, 0 : nq + 1, rsl], st[:, 0 : nq + 1, rsl], rot
                                )
                            nc.sync.dma_start(
                                qkv_sp[:, :, tsl].rearrange("m p t -> p m t"), st
                            )

                if 2 in phases:
                    # ---------------- phase 2: attention ----------------
                    with (
                        tc.tile_pool(name="attn", bufs=1) as attn_pool,
                        tc.tile_pool(name="kv", bufs=2) as kv_pool,
                        tc.tile_pool(name="vn", bufs=2) as vn_pool,
                        tc.tile_pool(name="qh", bufs=3) as q_pool,
                        tc.tile_pool(name="pt", bufs=12) as pt_pool,
                        tc.tile_pool(name="sm", bufs=4) as sm_pool,
                    ):
                        attnT = attn_pool.tile([128, B * nq, S], MDT)
                        for b in range(B):
                            bsl = ds(b * S, S)
                            kT = kv_pool.tile([128, S], MDT, tag="kT")
                            nc.sync.dma_start(kT, qkv_sp[nq, :, bsl])
                            vT = kv_pool.tile([128, S], MDT, tag="vT")
                            nc.sync.dma_start(vT, qkv_sp[nq + 1, :, bsl])
                            vn = vn_pool.tile([128, NKT, 128], MDT)
                            for kt in range(NKT):
                                tp = psum.tile([128, 128], MDT, tag="bank")
                                nc.tensor.transpose(tp, vT[:, ts(kt, 128)], ident)
                                nc.vector.tensor_copy(vn[:, kt, :], tp)
                            for h in range(nq):
                                q = q_pool.tile([128, S], MDT)
                                nc.sync.dma_start(q, qkv_sp[h, :, bsl])
                                for j in range(NJ):
                                    nkt = (j + 1) * NRT
                                    qsl = ds(j * QW, QW)
                                    pts = []
                                    for kt in range(nkt):
                                        sp = psum.tile([128, QW], F32, tag="bank")
                                        nc.tensor.matmul(
                                            sp,
                                            kT[:, ts(kt, 128)],
                                            q[:, qsl],
                                            start=True,
                                            stop=True,
                                        )
                                        pt = pt_pool.tile([128, QW], MDT, tag="pt")
                                        nc.scalar.activation(pt, sp, EXPF, bias=nbias[:, 0:1], scale=scale)
                                        if kt >= j * NRT:
                                            nc.vector.tensor_mul(
                                                pt, pt, masks_sb[:, kt - j * NRT, :]
                                            )
                                        pts.append(pt)
                                    sum_ps = psum.tile([1, QW], F32, tag="bank")
                                    for i, pt in enumerate(pts):
                                        nc.tensor.matmul(
                                            sum_ps,
                                            ones,
                                            pt,
                                            start=(i == 0),
                                            stop=(i == len(pts) - 1),
                                        )
                                    recip = sm_pool.tile([1, QW], F32, tag="recip")
                                    nc.vector.reciprocal(recip, sum_ps)
                                    rb = sm_pool.tile([128, QW], F32, tag="rb")
                                    nc.gpsimd.partition_broadcast(rb, recip)
                                    pv = psum.tile([128, QW], F32, tag="bank")
                                    for i, pt in enumerate(pts):
                                        nc.tensor.matmul(
                                            pv,
                                            vn[:, i, :],
                                            pt,
                                            start=(i == 0),
                                            stop=(i == len(pts) - 1),
                                        )
                                    nc.vector.tensor_mul(
                                        attnT[:, b * nq + h, ds(j * QW, QW)], pv, rb
                                    )

                        if 3 in phases:
                            # ---------------- phase 3: o_proj ----------------
                            with (
                                tc.tile_pool(name="wo", bufs=2) as wo_pool,
                                tc.tile_pool(name="ot", bufs=4) as ot_pool,
                            ):
                                for nh in range(H // 512):
                                    hsl = ds(nh * 512, 512)
                                    wo_t = wo_pool.tile([128, nq, 512], MDT)
                                    nc.sync.dma_start(
                                        wo_t, woT[:, :, hsl].rearrange("a p n -> p a n")
                                    )
                                    for tm in range(T // 128):
                                        b = (tm * 128) // S
                                        tloc = tm * 128 - b * S
                                        ps = psum.tile([128, 512], F32, tag="bank")
                                        for a in range(nq):
                                            nc.tensor.matmul(
                                                ps,
                                                attnT[:, b * nq + a, ds(tloc, 128)],
                                                wo_t[:, a, :],
                                                start=(a == 0),
                                                stop=(a == nq - 1),
                                            )
                                        ot = ot_pool.tile([128, 512], F16)
                                        if tm % 2 == 0:
                                            nc.scalar.copy(ot, ps)
                                        else:
                                            nc.vector.tensor_copy(ot, ps)
                                        nc.sync.dma_start(out[ts(tm, 128), hsl], ot)
    nc.compile()
    return nc


def prep_core_inputs(cfg: Cfg, hidden, w_qkv, w_o, core: int):
    """Build the per-core input map (C-contiguous, matmul-path dtype)."""
    ndt = np.float16 if cfg.mm16 else np.float32
    T, H, S, nq, D = cfg.T, cfg.H, cfg.S, cfg.nq, cfg.D
    NQ_TOT = cfg.n_cores * nq
    xT = np.ascontiguousarray(hidden.T.astype(ndt)).reshape(cfg.KO, 128, T)
    q0 = core * nq * D
    rows = list(range(q0, q0 + nq * D))
    rows += list(range(NQ_TOT * D + core * D, NQ_TOT * D + (core + 1) * D))
    kv_heads = cfg.n_cores  # one kv head per core
    rows += list(
        range((NQ_TOT + kv_heads) * D + core * D, (NQ_TOT + kv_heads) * D + (core + 1) * D)
    )
    wqkvT = np.ascontiguousarray(w_qkv[rows, :].T.astype(ndt)).reshape(cfg.KO, 128, cfg.NM * 128)
    woT = np.ascontiguousarray(
        w_o[:, core * nq * D : (core + 1) * nq * D].T.astype(ndt)
    ).reshape(nq, 128, H)
    return {"xT": xT, "wqkvT": wqkvT, "woT": woT}


def prep_shared_inputs(cfg: Cfg, rope_theta=10000.0):
    ndt = np.float16 if cfg.mm16 else np.float32
    S, D = cfg.S, cfg.D
    NRT = cfg.QW // 128
    inv = 1.0 / (rope_theta ** (np.arange(0, D, 2, dtype=np.float64) / D))
    ang = np.arange(S, dtype=np.float64)[:, None] * inv[None, :]  # [S, 64]
    cos = np.cos(ang).T.astype(np.float32)  # [64, S]
    sin = np.sin(ang).T.astype(np.float32)
    cosT = np.concatenate([cos, cos], axis=0)  # [128, S]
    sinT = np.concatenate([-sin, sin], axis=0)
    kl = np.arange(128)[:, None, None]
    r = np.arange(NRT)[None, :, None]
    ql = np.arange(cfg.QW)[None, None, :]
    masksT = ((r * 128 + kl) <= ql).astype(np.float32)
    return {
        "cosT": np.ascontiguousarray(cosT.astype(ndt)),
        "sinT": np.ascontiguousarray(sinT.astype(ndt)),
        "masksT": np.ascontiguousarray(masksT.astype(ndt)),
        "identT": np.eye(128, dtype=ndt),
        "onesT": np.ones((128, 1), dtype=ndt),
    }


_CACHE = {}
LAST_EXEC_NS = None


def _get_nc(cfg: Cfg) -> bass.Bass:
    if cfg not in _CACHE:
        _CACHE[cfg] = build_nc(cfg)
    return _CACHE[cfg]


def kernel(hidden_states=None, w_qkv=None, w_o=None, seq_len=None, **_):
    cfg = FULL
    hidden = np.asarray(hidden_states, dtype=np.float32)
    w_qkv = np.asarray(w_qkv, dtype=np.float32)
    w_o = np.asarray(w_o, dtype=np.float32)

    nc = _get_nc(cfg)
    shared = prep_shared_inputs(cfg)
    in_maps = []
    for c in range(cfg.n_cores):
        m = dict(shared)
        m.update(prep_core_inputs(cfg, hidden, w_qkv, w_o, c))
        in_maps.append(m)

    trace = os.environ.get("KERNEL_TRACE", "0") == "1"
    res = run_bass_kernel_spmd(
        nc, in_maps, core_ids=list(range(cfg.n_cores)), trace=trace
    )
    global LAST_EXEC_NS
    if res.exec_time_ns is not None:
        LAST_EXEC_NS = res.exec_time_ns
        print(f"HW exec time: {res.exec_time_ns} ns")
        if res.instructions_and_trace is not None:
            print(f"trace: {res.instructions_and_trace[1]}")
    outs = [r["o_partial"] for r in res.results]
    acc = np.zeros(outs[0].shape, dtype=np.float32)
    for o in outs:
        acc += o.astype(np.float32)
    return acc



# revision 4
# speedup vs baseline: 1.3296x; 1.0023x over previous
# Mistral-style GQA attention layer (QKV proj + RoPE + causal attention +
# o_proj), tensor-parallel over heads across 8 NeuronCores.  v3: bf16
# matmul path, fused per-batch pipeline (no DRAM spill), softmax
# denominator via all-ones matmul (reduce+broadcast in one PE op),
# o_proj blocks of batch b-1 interleaved into batch b's attention to
# fill dependency stalls.
#
# Sharding (8-way TP over heads): core c owns q heads [4c..4c+4) and kv head c.
#   - w_qkv rows sharded: 4 q-head blocks + 1 k block + 1 v block per core
#   - w_o columns sharded: each core computes a partial o_proj output,
#     host sums the 8 partials (the "all-reduce").

import os

import numpy as np
import ml_dtypes

import concourse.bass as bass
from concourse import bacc
import concourse.mybir as mybir
import concourse.tile as tile
from concourse.bass import ds, ts
from concourse.bass_utils import run_bass_kernel_spmd

F32 = mybir.dt.float32
BF16 = mybir.dt.bfloat16
F16 = mybir.dt.float16
EXPF = mybir.ActivationFunctionType.Exp

T = 4096
H = 4096
S = 1024
B = 4
NQ = 4          # q heads per core
NM = NQ + 2     # qkv row-tiles per core (4 q + 1 k + 1 v)
D = 128
KO = H // 128   # contraction tiles
NKT = S // 128  # key tiles per batch
N_CORES = 8
SCALE = 1.0 / np.sqrt(D)
KH = 8          # x k-tiles per DMA chunk
NCH = KO // KH


def build_nc() -> bass.Bass:
    nc = bacc.Bacc("TRN2", target_bir_lowering=False, debug=False, num_devices=N_CORES)

    xh = nc.dram_tensor("xh", [128, KO, T], BF16, kind="ExternalInput")
    wqkvh = nc.dram_tensor("wqkvh", [128, KO, NM * 128], BF16, kind="ExternalInput")
    woh = nc.dram_tensor("woh", [128, NQ, H], BF16, kind="ExternalInput")
    cosT = nc.dram_tensor("cosT", [128, S], BF16, kind="ExternalInput")
    sinT = nc.dram_tensor("sinT", [128, S], BF16, kind="ExternalInput")
    masksT = nc.dram_tensor("masksT", [128, 4, 512], BF16, kind="ExternalInput")
    identT = nc.dram_tensor("identT", [128, 128], BF16, kind="ExternalInput")
    onesT = nc.dram_tensor("onesT", [128, 128], BF16, kind="ExternalInput")
    out = nc.dram_tensor("o_partial", [T, H], F16, kind="ExternalOutput")

    with tile.TileContext(nc) as tc:
        with (
            tc.tile_pool(name="psum", bufs=8, space="PSUM") as psum,
            tc.tile_pool(name="consts", bufs=1) as consts,
            tc.tile_pool(name="wq", bufs=1) as wq_pool,
            tc.tile_pool(name="xin", bufs=2) as xin,
            tc.tile_pool(name="rot", bufs=2) as rot_pool,
            tc.tile_pool(name="qkv", bufs=2) as qkv_pool,
            tc.tile_pool(name="attn", bufs=2) as attn_pool,
            tc.tile_pool(name="vn", bufs=2) as vn_pool,
            tc.tile_pool(name="pt", bufs=8) as pt_pool,
            tc.tile_pool(name="sm", bufs=3) as sm_pool,
            tc.tile_pool(name="ot", bufs=2) as ot_pool,
        ):
            ident = consts.tile([128, 128], BF16)
            nc.sync.dma_start(ident, identT[:])
            ones = consts.tile([128, 128], BF16)
            nc.sync.dma_start(ones, onesT[:])
            masks = consts.tile([128, 4, 512], BF16)
            nc.sync.dma_start(masks, masksT[:])
            cos_sb = consts.tile([128, S], BF16)
            nc.sync.dma_start(cos_sb, cosT[:])
            sin_sb = consts.tile([128, S], BF16)
            nc.sync.dma_start(sin_sb, sinT[:])
            nbias = consts.tile([128, 1], F32)
            nc.gpsimd.memset(nbias, -4.0)

            # weights resident in SBUF; w chunks so phase A can start early
            w_all = wq_pool.tile([128, KO, NM * 128], BF16)
            for ch in range(NCH):
                nc.sync.dma_start(
                    w_all[:, ds(ch * KH, KH), :], wqkvh[:, ds(ch * KH, KH), :]
                )
            wo = wq_pool.tile([128, NQ, H], BF16)
            nc.sync.dma_start(wo, woh[:])

            attn_tiles = {}

            def emit_oproj_block(b, tm):
                """o_proj for token tile tm of batch b: 8 x [128,512] out cols.

                Evacuation is all-DVE: scalar must stay free for attention
                EXPs these blocks are interleaved with.
                """
                attn_b = attn_tiles[b]
                ot = ot_pool.tile([128, H], F16, tag="ot", name="ot")
                for hs in range(8):
                    po = psum.tile([128, 512], F32, tag="sp", name="po", bufs=4)
                    for a in range(NQ):
                        nc.tensor.matmul(
                            po,
                            attn_b[:, a, ts(tm, 128)],
                            wo[:, a, ds(hs * 512, 512)],
                            start=(a == 0),
                            stop=(a == NQ - 1),
                        )
                    nc.vector.tensor_copy(ot[:, ds(hs * 512, 512)], po)
                nc.sync.dma_start(out[ds(b * S + tm * 128, 128), :], ot)

            def emit_attn_head(qkv_b, vn, attn_b, j, h):
                nkt = 4 * (j + 1)
                qs = qkv_b[:, h, ds(j * 512, 512)]
                pv = psum.tile([128, 512], F32, tag="pv", name="pv", bufs=1)
                acc = sm_pool.tile([128, 512], BF16, tag="acc", name="acc")
                pts = []
                for half in range(j + 1):
                    for kk in range(4):
                        kt = half * 4 + kk
                        sp = psum.tile([128, 512], F32, tag="sp", name="sp", bufs=4)
                        nc.tensor.matmul(
                            sp, qkv_b[:, NQ, ts(kt, 128)], qs, start=True, stop=True
                        )
                        pt = pt_pool.tile([128, 512], BF16, tag="pt", name="pt")
                        nc.scalar.activation(
                            pt, sp, EXPF, bias=nbias[:, 0:1], scale=SCALE
                        )
                        if kt >= j * 4:
                            nc.vector.tensor_mul(pt, pt, masks[:, kt - j * 4, :])
                        if kt == 0:
                            nc.vector.tensor_copy(acc, pt)
                        else:
                            nc.vector.tensor_add(acc, acc, pt)
                        pts.append(pt)
                    for kk in range(4):
                        kt = half * 4 + kk
                        nc.tensor.matmul(
                            pv,
                            vn[:, kt, :],
                            pts[kt],
                            start=(kt == 0),
                            stop=(kt == nkt - 1),
                        )
                # denominator: all-ones matmul = partition-reduce + broadcast
                rbm = psum.tile([128, 512], F32, tag="sp", name="rbm", bufs=4)
                nc.tensor.matmul(rbm, ones, acc, start=True, stop=True)
                recip = sm_pool.tile([128, 512], F32, tag="recip", name="recip")
                nc.vector.reciprocal_approx_fast(recip, rbm)
                nc.vector.tensor_mul(attn_b[:, h, ds(j * 512, 512)], pv, recip)

            for b in range(B):
                # ---------------- A(b): QKV projection + RoPE ----------------
                # 256-token slabs: the 6 fp32 accumulators fit 3 PSUM banks,
                # leaving 4 ("sp") + 1 ("pv") banks for the attention/o_proj
                # work this phase overlaps with.
                qkv_b = qkv_pool.tile([128, NM, S], BF16, tag="qkv", name="qkv_b")
                SLAB = 256
                XH = KO // 2  # x k-tiles per half-slab tile
                for sl in range(S // SLAB):
                    tok0 = b * S + sl * SLAB
                    tq = ds(sl * SLAB, SLAB)
                    # whole x slab resident; each m-group streams it again
                    xts = []
                    for hf in range(2):
                        xt = xin.tile([128, XH, SLAB], BF16, tag="xh", name="xt", bufs=3)
                        nc.sync.dma_start(xt, xh[:, ds(hf * XH, XH), ds(tok0, SLAB)])
                        xts.append(xt)
                    for g in range(2):
                        pss = [
                            psum.tile(
                                [128, SLAB], F32, tag="pa", name=f"qk_ps{mi}", bufs=3
                            )
                            for mi in range(3)
                        ]
                        for hf in range(2):
                            for k in range(XH):
                                for mi in range(3):
                                    nc.tensor.matmul(
                                        pss[mi],
                                        w_all[:, hf * XH + k, ts(g * 3 + mi, 128)],
                                        xts[hf][:, k, :],
                                        start=(hf == 0 and k == 0),
                                        stop=(hf == 1 and k == XH - 1),
                                    )
                        for mi in range(3):
                            nc.scalar.copy(qkv_b[:, g * 3 + mi, tq], pss[mi])
                    # RoPE on q heads + k head (rows 0..4), not v
                    rot = rot_pool.tile([128, NQ + 1, SLAB], BF16, tag="rot", name="rot")
                    nc.sync.dma_start(rot[0:64], qkv_b[64:128, 0 : NQ + 1, tq])
                    nc.sync.dma_start(rot[64:128], qkv_b[0:64, 0 : NQ + 1, tq])
                    s0 = sl * SLAB
                    cos_br = cos_sb[:, None, ds(s0, SLAB)].to_broadcast(
                        (128, NQ + 1, SLAB)
                    )
                    sin_br = sin_sb[:, None, ds(s0, SLAB)].to_broadcast(
                        (128, NQ + 1, SLAB)
                    )
                    nc.vector.tensor_mul(
                        qkv_b[:, 0 : NQ + 1, tq], qkv_b[:, 0 : NQ + 1, tq], cos_br
                    )
                    nc.vector.tensor_mul(rot, rot, sin_br)
                    nc.vector.tensor_add(
                        qkv_b[:, 0 : NQ + 1, tq], qkv_b[:, 0 : NQ + 1, tq], rot
                    )

                # V transposes (only need the V row of qkv_b — no RoPE dep)
                vn = vn_pool.tile([128, NKT, 128], BF16, tag="vn", name="vn")
                for kt in range(NKT):
                    tp = psum.tile([128, 128], BF16, tag="sp", name="tp", bufs=4)
                    nc.tensor.transpose(tp, qkv_b[:, NQ + 1, ts(kt, 128)], ident)
                    nc.vector.tensor_copy(vn[:, kt, :], tp)

                attn_b = attn_pool.tile([128, NQ, S], BF16, tag="attn", name="attn_b")
                attn_tiles[b] = attn_b

                # ---- B(b) attention, with C(b-1) o_proj blocks interleaved
                fill = list(range(8)) if b > 0 else []
                if fill:  # cover the RoPE tail of A(b) before the first head
                    emit_oproj_block(b - 1, fill.pop(0))
                    emit_oproj_block(b - 1, fill.pop(0))
                for j in range(2):
                    for h in range(NQ):
                        emit_attn_head(qkv_b, vn, attn_b, j, h)
                        if fill:
                            emit_oproj_block(b - 1, fill.pop(0))

            for tm in range(8):
                emit_oproj_block(B - 1, tm)
    nc.compile()
    return nc


def prep_core_inputs(hidden_bf, w_qkv, w_o, core: int):
    """Per-core input map; all matmul-path tensors pre-laid-out on host."""
    bf = ml_dtypes.bfloat16
    NQ_TOT = N_CORES * NQ
    q0 = core * NQ * D
    rows = list(range(q0, q0 + NQ * D))
    rows += list(range(NQ_TOT * D + core * D, NQ_TOT * D + (core + 1) * D))
    rows += list(
        range((NQ_TOT + N_CORES) * D + core * D, (NQ_TOT + N_CORES) * D + (core + 1) * D)
    )
    wqkvh = np.ascontiguousarray(
        w_qkv[rows, :].T.astype(bf).reshape(KO, 128, NM * 128).transpose(1, 0, 2)
    )
    woh = np.ascontiguousarray(
        w_o[:, core * NQ * D : (core + 1) * NQ * D]
        .T.astype(bf)
        .reshape(NQ, 128, H)
        .transpose(1, 0, 2)
    )
    return {"xh": hidden_bf, "wqkvh": wqkvh, "woh": woh}


def prep_shared_inputs(rope_theta=10000.0):
    bf = ml_dtypes.bfloat16
    inv = 1.0 / (rope_theta ** (np.arange(0, D, 2, dtype=np.float64) / D))
    ang = np.arange(S, dtype=np.float64)[:, None] * inv[None, :]  # [S, 64]
    cos = np.cos(ang).T.astype(np.float32)  # [64, S]
    sin = np.sin(ang).T.astype(np.float32)
    cosT = np.concatenate([cos, cos], axis=0)  # [128, S]
    sinT = np.concatenate([-sin, sin], axis=0)
    kl = np.arange(128)[:, None, None]
    r = np.arange(4)[None, :, None]
    ql = np.arange(512)[None, None, :]
    masksT = ((r * 128 + kl) <= ql).astype(np.float32)
    return {
        "cosT": np.ascontiguousarray(cosT.astype(bf)),
        "sinT": np.ascontiguousarray(sinT.astype(bf)),
        "masksT": np.ascontiguousarray(masksT.astype(bf)),
        "identT": np.eye(128, dtype=bf),
        "onesT": np.ones((128, 128), dtype=bf),
    }


_CACHE = {}
LAST_EXEC_NS = None


def kernel(hidden_states=None, w_qkv=None, w_o=None, seq_len=None, **_):
    bf = ml_dtypes.bfloat16
    hidden = np.asarray(hidden_states, dtype=np.float32)
    w_qkv = np.asarray(w_qkv, dtype=np.float32)
    w_o = np.asarray(w_o, dtype=np.float32)

    if "nc" not in _CACHE:
        _CACHE["nc"] = build_nc()
    nc = _CACHE["nc"]

    # x^T laid out [128, KO, T] once, shared across cores
    hidden_bf = np.ascontiguousarray(
        hidden.T.astype(bf).reshape(KO, 128, T).transpose(1, 0, 2)
    )
    shared = prep_shared_inputs()
    in_maps = []
    for c in range(N_CORES):
        m = dict(shared)
        m.update(prep_core_inputs(hidden_bf, w_qkv, w_o, c))
        in_maps.append(m)

    trace = os.environ.get("KERNEL_TRACE", "0") == "1"
    res = run_bass_kernel_spmd(
        nc, in_maps, core_ids=list(range(N_CORES)), trace=trace
    )
    global LAST_EXEC_NS
    if res.exec_time_ns is not None:
        LAST_EXEC_NS = res.exec_time_ns
        print(f"HW exec time: {res.exec_time_ns} ns")
        if res.instructions_and_trace is not None:
            print(f"trace: {res.instructions_and_trace[1]}")
    outs = [r["o_partial"] for r in res.results]
    acc = np.zeros(outs[0].shape, dtype=np.float32)
    for o in outs:
        acc += o.astype(np.float32)
    return acc


# revision 6
# speedup vs baseline: 1.3374x; 1.0059x over previous
# Mistral-style GQA attention layer (QKV proj + RoPE + causal attention +
# o_proj), tensor-parallel over heads across 8 NeuronCores.  v3: bf16
# matmul path, fused per-batch pipeline (no DRAM spill), softmax
# denominator via all-ones matmul (reduce+broadcast in one PE op),
# o_proj blocks of batch b-1 interleaved into batch b's attention to
# fill dependency stalls.
#
# Sharding (8-way TP over heads): core c owns q heads [4c..4c+4) and kv head c.
#   - w_qkv rows sharded: 4 q-head blocks + 1 k block + 1 v block per core
#   - w_o columns sharded: each core computes a partial o_proj output,
#     host sums the 8 partials (the "all-reduce").

import os
import sys
import types

import numpy as np
import ml_dtypes

import concourse.bass as bass


def _ensure_ntff_hook():
    """Best-effort: provide antenv.axon_hooks if the image lacks it.

    concourse's trace path does `from antenv.axon_hooks import ...` when
    tracing is requested (e.g. BASS_TRACE=1); on images without that
    module the import raises instead of degrading.  Recreate the hook via
    trn_agent_boot's ctypes shim when possible, else register a None-hook
    so bass_utils falls back to running without a trace.
    """
    try:
        import antenv.axon_hooks  # noqa: F401
        return
    except ImportError:
        pass
    try:
        hook = None
        try:
            from trn_agent_boot import trn_boot

            so = "/opt/axon/libaxon_pjrt.so"
            if os.path.exists(so):
                hook = trn_boot._ntff_profile_via_ctypes(so)
        except Exception:
            hook = None
        mod = types.ModuleType("antenv.axon_hooks")
        store = {"h": hook}
        mod.get_axon_ntff_profile_hook = lambda: store["h"]
        mod.set_axon_ntff_profile_hook = lambda h: store.__setitem__("h", h)
        sys.modules["antenv.axon_hooks"] = mod
    except Exception:
        pass


_ensure_ntff_hook()
from concourse import bacc
import concourse.mybir as mybir
import concourse.tile as tile
from concourse.bass import ds, ts
from concourse.bass_utils import run_bass_kernel_spmd

F32 = mybir.dt.float32
BF16 = mybir.dt.bfloat16
F16 = mybir.dt.float16
EXPF = mybir.ActivationFunctionType.Exp

T = 4096
H = 4096
S = 1024
B = 4
NQ = 4          # q heads per core
NM = NQ + 2     # qkv row-tiles per core (4 q + 1 k + 1 v)
D = 128
KO = H // 128   # contraction tiles
NKT = S // 128  # key tiles per batch
N_CORES = 8
SCALE = 1.0 / np.sqrt(D)
KH = 8          # x k-tiles per DMA chunk
NCH = KO // KH


def build_nc() -> bass.Bass:
    nc = bacc.Bacc("TRN2", target_bir_lowering=False, debug=False, num_devices=N_CORES)

    xh = nc.dram_tensor("xh", [T // 256, 128, KO * 256], BF16, kind="ExternalInput")
    wqkvh = nc.dram_tensor("wqkvh", [128, KO, NM * 128], BF16, kind="ExternalInput")
    woh = nc.dram_tensor("woh", [128, NQ, H], BF16, kind="ExternalInput")
    cosT = nc.dram_tensor("cosT", [128, S], BF16, kind="ExternalInput")
    sinT = nc.dram_tensor("sinT", [128, S], BF16, kind="ExternalInput")
    masksT = nc.dram_tensor("masksT", [128, 4, 512], BF16, kind="ExternalInput")
    identT = nc.dram_tensor("identT", [128, 128], BF16, kind="ExternalInput")
    onesT = nc.dram_tensor("onesT", [128, 128], BF16, kind="ExternalInput")
    out = nc.dram_tensor("o_partial", [T, H], F16, kind="ExternalOutput")

    with tile.TileContext(nc) as tc:
        with (
            tc.tile_pool(name="psum", bufs=8, space="PSUM") as psum,
            tc.tile_pool(name="consts", bufs=1) as consts,
            tc.tile_pool(name="wq", bufs=1) as wq_pool,
            tc.tile_pool(name="xin", bufs=2) as xin,
            tc.tile_pool(name="rot", bufs=2) as rot_pool,
            tc.tile_pool(name="qkv", bufs=2) as qkv_pool,
            tc.tile_pool(name="attn", bufs=2) as attn_pool,
            tc.tile_pool(name="vn", bufs=2) as vn_pool,
            tc.tile_pool(name="pt", bufs=8) as pt_pool,
            tc.tile_pool(name="sm", bufs=3) as sm_pool,
            tc.tile_pool(name="ot", bufs=2) as ot_pool,
        ):
            attn_tiles = {}
            pending_tail = None
            xt_map = {}

            def load_x(bb, sl):
                xts = []
                for hf in range(2):
                    xt = xin.tile(
                        [128, (KO // 2) * 256], BF16, tag="xh", name="xt", bufs=4
                    )
                    nc.sync.dma_start(
                        xt,
                        xh[
                            (bb * S + sl * 256) // 256,
                            :,
                            ds(hf * (KO // 2) * 256, (KO // 2) * 256),
                        ],
                    )
                    xts.append(xt)
                xt_map[(bb, sl)] = xts

            # weights ride the scalar-engine HWDGE ring so the sync ring can
            # stream x tiles immediately (startup would otherwise head-block
            # ~35us behind 10MB of weights).
            w_all = wq_pool.tile([128, KO, NM * 128], BF16)
            for ch in range(NCH):
                nc.scalar.dma_start(
                    w_all[:, ds(ch * KH, KH), :], wqkvh[:, ds(ch * KH, KH), :]
                )
            load_x(0, 0)
            cos_sb = consts.tile([128, S], BF16)
            nc.sync.dma_start(cos_sb, cosT[:])
            sin_sb = consts.tile([128, S], BF16)
            nc.sync.dma_start(sin_sb, sinT[:])
            ident = consts.tile([128, 128], BF16)
            nc.scalar.dma_start(ident, identT[:])
            ones = consts.tile([128, 128], BF16)
            nc.scalar.dma_start(ones, onesT[:])
            masks = consts.tile([128, 4, 512], BF16)
            nc.scalar.dma_start(masks, masksT[:])
            nbias = consts.tile([128, 1], F32)
            nc.gpsimd.memset(nbias, -4.0)
            wo = wq_pool.tile([128, NQ, H], BF16)
            nc.scalar.dma_start(wo, woh[:])

            def emit_oproj_block(b, tm):
                """o_proj for token tile tm of batch b: 8 x [128,512] out cols.

                Evacuation is all-DVE: scalar must stay free for attention
                EXPs these blocks are interleaved with.
                """
                attn_b = attn_tiles[b]
                ot = ot_pool.tile([128, H], F16, tag="ot", name="ot")
                for hs in range(8):
                    po = psum.tile([128, 512], F32, tag="sp", name="po", bufs=4)
                    for a in range(NQ):
                        nc.tensor.matmul(
                            po,
                            attn_b[:, a, ts(tm, 128)],
                            wo[:, a, ds(hs * 512, 512)],
                            start=(a == 0),
                            stop=(a == NQ - 1),
                        )
                    nc.vector.tensor_copy(ot[:, ds(hs * 512, 512)], po)
                    if hs == 3:
                        nc.sync.dma_start(
                            out[ds(b * S + tm * 128, 128), 0:2048], ot[:, 0:2048]
                        )
                nc.sync.dma_start(
                    out[ds(b * S + tm * 128, 128), 2048:4096], ot[:, 2048:4096]
                )

            def emit_attn_head(qkv_b, vn, attn_b, j, h, defer_tail=False):
                nkt = 4 * (j + 1)
                qs = qkv_b[:, h, ds(j * 512, 512)]
                pv = psum.tile([128, 512], F32, tag="pv", name="pv", bufs=1)
                acc = sm_pool.tile([128, 512], BF16, tag="acc", name="acc")
                pts = []
                for half in range(j + 1):
                    for kk in range(4):
                        kt = half * 4 + kk
                        sp = psum.tile([128, 512], F32, tag="sp", name="sp", bufs=4)
                        nc.tensor.matmul(
                            sp, qkv_b[:, NQ, ts(kt, 128)], qs, start=True, stop=True
                        )
                        pt = pt_pool.tile([128, 512], BF16, tag="pt", name="pt")
                        nc.scalar.activation(
                            pt, sp, EXPF, bias=nbias[:, 0:1], scale=SCALE
                        )
                        if kt >= j * 4:
                            nc.vector.tensor_mul(pt, pt, masks[:, kt - j * 4, :])
                        if kt == 0:
                            nc.vector.tensor_copy(acc, pt)
                        else:
                            nc.vector.tensor_add(acc, acc, pt)
                        pts.append(pt)
                    for kk in range(4):
                        kt = half * 4 + kk
                        nc.tensor.matmul(
                            pv,
                            vn[:, kt, :],
                            pts[kt],
                            start=(kt == 0),
                            stop=(kt == nkt - 1),
                        )

                def tail():
                    # denominator: all-ones matmul = partition-reduce+broadcast
                    rbm = psum.tile([128, 512], F32, tag="sp", name="rbm", bufs=4)
                    nc.tensor.matmul(rbm, ones, acc, start=True, stop=True)
                    recip = sm_pool.tile([128, 512], F32, tag="recip", name="recip")
                    nc.vector.reciprocal_approx_fast(recip, rbm)
                    nc.vector.tensor_mul(attn_b[:, h, ds(j * 512, 512)], pv, recip)

                if defer_tail:
                    return tail
                tail()
                return None

            for b in range(B):
                # ---------------- A(b): QKV projection + RoPE ----------------
                # 256-token slabs: the 6 fp32 accumulators fit 3 PSUM banks,
                # leaving 4 ("sp") + 1 ("pv") banks for the attention/o_proj
                # work this phase overlaps with.
                qkv_b = qkv_pool.tile([128, NM, S], BF16, tag="qkv", name="qkv_b")
                SLAB = 256
                XH = KO // 2  # x k-tiles per half-slab tile
                for sl in range(S // SLAB):
                    tok0 = b * S + sl * SLAB
                    tq = ds(sl * SLAB, SLAB)
                    # whole x slab resident; each m-group streams it again
                    if (b, sl) not in xt_map:
                        load_x(b, sl)
                    xts = xt_map.pop((b, sl))
                    if sl == 1 and pending_tail is not None:
                        # flush the previous batch's last softmax tail behind
                        # this batch's first slab of projection matmuls
                        pending_tail()
                        pending_tail = None
                    for g in range(2):
                        pss = [
                            psum.tile(
                                [128, SLAB], F32, tag="pa", name=f"qk_ps{mi}", bufs=3
                            )
                            for mi in range(3)
                        ]
                        for hf in range(2):
                            for k in range(XH):
                                for mi in range(3):
                                    nc.tensor.matmul(
                                        pss[mi],
                                        w_all[:, hf * XH + k, ts(g * 3 + mi, 128)],
                                        xts[hf][:, ds(k * SLAB, SLAB)],
                                        start=(hf == 0 and k == 0),
                                        stop=(hf == 1 and k == XH - 1),
                                    )
                        for mi in range(3):
                            nc.scalar.copy(qkv_b[:, g * 3 + mi, tq], pss[mi])
                    # RoPE on q heads + k head (rows 0..4), not v
                    rot = rot_pool.tile([128, NQ + 1, SLAB], BF16, tag="rot", name="rot")
                    nc.scalar.dma_start(rot[0:64], qkv_b[64:128, 0 : NQ + 1, tq])
                    nc.scalar.dma_start(rot[64:128], qkv_b[0:64, 0 : NQ + 1, tq])
                    s0 = sl * SLAB
                    cos_br = cos_sb[:, None, ds(s0, SLAB)].to_broadcast(
                        (128, NQ + 1, SLAB)
                    )
                    sin_br = sin_sb[:, None, ds(s0, SLAB)].to_broadcast(
                        (128, NQ + 1, SLAB)
                    )
                    nc.vector.tensor_mul(
                        qkv_b[:, 0 : NQ + 1, tq], qkv_b[:, 0 : NQ + 1, tq], cos_br
                    )
                    nc.vector.tensor_mul(rot, rot, sin_br)
                    nc.vector.tensor_add(
                        qkv_b[:, 0 : NQ + 1, tq], qkv_b[:, 0 : NQ + 1, tq], rot
                    )

                if b + 1 < B:
                    # prefetch next batch's first x slab: its DMA issue must
                    # not queue behind B(b)'s output-DMA issues on this ring
                    load_x(b + 1, 0)

                # V transposes (only need the V row of qkv_b — no RoPE dep)
                vn = vn_pool.tile([128, NKT, 128], BF16, tag="vn", name="vn")
                for kt in range(NKT):
                    tp = psum.tile([128, 128], BF16, tag="sp", name="tp", bufs=4)
                    nc.tensor.transpose(tp, qkv_b[:, NQ + 1, ts(kt, 128)], ident)
                    nc.vector.tensor_copy(vn[:, kt, :], tp)

                attn_b = attn_pool.tile([128, NQ, S], BF16, tag="attn", name="attn_b")
                attn_tiles[b] = attn_b

                # ---- B(b) attention, with C(b-1) o_proj blocks interleaved
                fill = list(range(8)) if b > 0 else []
                if fill:  # cover the RoPE tail of A(b) before the first head
                    emit_oproj_block(b - 1, fill.pop(0))
                    emit_oproj_block(b - 1, fill.pop(0))
                for j in range(2):
                    for h in range(NQ):
                        defer = j == 1 and h == NQ - 1
                        t = emit_attn_head(qkv_b, vn, attn_b, j, h, defer_tail=defer)
                        if defer:
                            pending_tail = t
                        if fill:
                            emit_oproj_block(b - 1, fill.pop(0))

            if pending_tail is not None:
                pending_tail()
                pending_tail = None
            for tm in range(8):
                emit_oproj_block(B - 1, tm)
    nc.compile()
    return nc


def prep_core_inputs(hidden_bf, w_qkv, w_o, core: int):
    """Per-core input map; all matmul-path tensors pre-laid-out on host."""
    bf = ml_dtypes.bfloat16
    NQ_TOT = N_CORES * NQ
    q0 = core * NQ * D
    rows = list(range(q0, q0 + NQ * D))
    rows += list(range(NQ_TOT * D + core * D, NQ_TOT * D + (core + 1) * D))
    rows += list(
        range((NQ_TOT + N_CORES) * D + core * D, (NQ_TOT + N_CORES) * D + (core + 1) * D)
    )
    wqkvh = np.ascontiguousarray(
        w_qkv[rows, :].T.astype(bf).reshape(KO, 128, NM * 128).transpose(1, 0, 2)
    )
    woh = np.ascontiguousarray(
        w_o[:, core * NQ * D : (core + 1) * NQ * D]
        .T.astype(bf)
        .reshape(NQ, 128, H)
        .transpose(1, 0, 2)
    )
    return {"xh": hidden_bf, "wqkvh": wqkvh, "woh": woh}


def prep_shared_inputs(rope_theta=10000.0):
    bf = ml_dtypes.bfloat16
    inv = 1.0 / (rope_theta ** (np.arange(0, D, 2, dtype=np.float64) / D))
    ang = np.arange(S, dtype=np.float64)[:, None] * inv[None, :]  # [S, 64]
    cos = np.cos(ang).T.astype(np.float32)  # [64, S]
    sin = np.sin(ang).T.astype(np.float32)
    cosT = np.concatenate([cos, cos], axis=0)  # [128, S]
    sinT = np.concatenate([-sin, sin], axis=0)
    kl = np.arange(128)[:, None, None]
    r = np.arange(4)[None, :, None]
    ql = np.arange(512)[None, None, :]
    masksT = ((r * 128 + kl) <= ql).astype(np.float32)
    return {
        "cosT": np.ascontiguousarray(cosT.astype(bf)),
        "sinT": np.ascontiguousarray(sinT.astype(bf)),
        "masksT": np.ascontiguousarray(masksT.astype(bf)),
        "identT": np.eye(128, dtype=bf),
        "onesT": np.ones((128, 128), dtype=bf),
    }


_CACHE = {}
LAST_EXEC_NS = None


def kernel(hidden_states=None, w_qkv=None, w_o=None, seq_len=None, **_):
    bf = ml_dtypes.bfloat16
    hidden = np.asarray(hidden_states, dtype=np.float32)
    w_qkv = np.asarray(w_qkv, dtype=np.float32)
    w_o = np.asarray(w_o, dtype=np.float32)

    if "nc" not in _CACHE:
        _CACHE["nc"] = build_nc()
    nc = _CACHE["nc"]

    # x^T laid out [128, KO, T] once, shared across cores
    hidden_bf = np.ascontiguousarray(
        hidden.T.astype(bf)
        .reshape(KO, 128, T // 256, 256)
        .transpose(2, 1, 0, 3)
        .reshape(T // 256, 128, KO * 256)
    )
    shared = prep_shared_inputs()
    in_maps = []
    for c in range(N_CORES):
        m = dict(shared)
        m.update(prep_core_inputs(hidden_bf, w_qkv, w_o, c))
        in_maps.append(m)

    trace = os.environ.get("KERNEL_TRACE", "0") == "1"
    res = run_bass_kernel_spmd(
        nc, in_maps, core_ids=list(range(N_CORES)), trace=trace
    )
    global LAST_EXEC_NS
    if res.exec_time_ns is not None:
        LAST_EXEC_NS = res.exec_time_ns
        print(f"HW exec time: {res.exec_time_ns} ns")
        if res.instructions_and_trace is not None:
            print(f"trace: {res.instructions_and_trace[1]}")
    outs = [r["o_partial"] for r in res.results]
    acc = np.zeros(outs[0].shape, dtype=np.float32)
    for o in outs:
        acc += o.astype(np.float32)
    return acc


# revision 7
# speedup vs baseline: 1.3427x; 1.0039x over previous
# Mistral-style GQA attention layer (QKV proj + RoPE + causal attention +
# o_proj), tensor-parallel over heads across 8 NeuronCores.  v3: bf16
# matmul path, fused per-batch pipeline (no DRAM spill), softmax
# denominator via all-ones matmul (reduce+broadcast in one PE op),
# o_proj blocks of batch b-1 interleaved into batch b's attention to
# fill dependency stalls.
#
# Sharding (8-way TP over heads): core c owns q heads [4c..4c+4) and kv head c.
#   - w_qkv rows sharded: 4 q-head blocks + 1 k block + 1 v block per core
#   - w_o columns sharded: each core computes a partial o_proj output,
#     host sums the 8 partials (the "all-reduce").

import os

import sys
import types

import numpy as np
import ml_dtypes

import concourse.bass as bass


def _ensure_ntff_hook():
    """Best-effort: provide antenv.axon_hooks if the image lacks it.

    concourse's trace path does `from antenv.axon_hooks import ...` when
    tracing is requested (e.g. BASS_TRACE=1); on images without that
    module the import raises instead of degrading.  Recreate the hook via
    trn_agent_boot's ctypes shim when possible, else register a None-hook
    so bass_utils falls back to running without a trace.
    """
    try:
        import antenv.axon_hooks  # noqa: F401
        return
    except ImportError:
        pass
    try:
        hook = None
        try:
            from trn_agent_boot import trn_boot

            so = "/opt/axon/libaxon_pjrt.so"
            if os.path.exists(so):
                hook = trn_boot._ntff_profile_via_ctypes(so)
        except Exception:
            hook = None
        mod = types.ModuleType("antenv.axon_hooks")
        store = {"h": hook}
        mod.get_axon_ntff_profile_hook = lambda: store["h"]
        mod.set_axon_ntff_profile_hook = lambda h: store.__setitem__("h", h)
        sys.modules["antenv.axon_hooks"] = mod
    except Exception:
        pass


_ensure_ntff_hook()
from concourse import bacc
import concourse.mybir as mybir
import concourse.tile as tile
from concourse.bass import ds, ts
from concourse.bass_utils import run_bass_kernel_spmd

F32 = mybir.dt.float32
BF16 = mybir.dt.bfloat16
F16 = mybir.dt.float16
EXPF = mybir.ActivationFunctionType.Exp

T = 4096
H = 4096
S = 1024
B = 4
NQ = 4          # q heads per core
NM = NQ + 2     # qkv row-tiles per core (4 q + 1 k + 1 v)
D = 128
KO = H // 128   # contraction tiles
NKT = S // 128  # key tiles per batch
N_CORES = 8
SCALE = 1.0 / np.sqrt(D)
KH = 8          # x k-tiles per DMA chunk
NCH = KO // KH


def build_nc() -> bass.Bass:
    nc = bacc.Bacc("TRN2", target_bir_lowering=False, debug=False, num_devices=N_CORES)

    xh = nc.dram_tensor("xh", [T // 256, 128, KO * 256], BF16, kind="ExternalInput")
    wqkvh = nc.dram_tensor("wqkvh", [128, KO, NM * 128], BF16, kind="ExternalInput")
    woh = nc.dram_tensor("woh", [128, NQ, H], BF16, kind="ExternalInput")
    cosT = nc.dram_tensor("cosT", [128, S], BF16, kind="ExternalInput")
    sinT = nc.dram_tensor("sinT", [128, S], BF16, kind="ExternalInput")
    masksT = nc.dram_tensor("masksT", [128, 4, 512], BF16, kind="ExternalInput")
    identT = nc.dram_tensor("identT", [128, 128], BF16, kind="ExternalInput")
    onesT = nc.dram_tensor("onesT", [128, 128], BF16, kind="ExternalInput")
    out = nc.dram_tensor("o_partial", [T, H], F16, kind="ExternalOutput")

    with tile.TileContext(nc) as tc:
        with (
            tc.tile_pool(name="psum", bufs=8, space="PSUM") as psum,
            tc.tile_pool(name="consts", bufs=1) as consts,
            tc.tile_pool(name="wq", bufs=1) as wq_pool,
            tc.tile_pool(name="xin", bufs=2) as xin,
            tc.tile_pool(name="rot", bufs=2) as rot_pool,
            tc.tile_pool(name="qkv", bufs=2) as qkv_pool,
            tc.tile_pool(name="attn", bufs=2) as attn_pool,
            tc.tile_pool(name="vn", bufs=2) as vn_pool,
            tc.tile_pool(name="pt", bufs=8) as pt_pool,
            tc.tile_pool(name="sm", bufs=3) as sm_pool,
            tc.tile_pool(name="ot", bufs=2) as ot_pool,
        ):
            attn_tiles = {}
            pending_tail = None
            xt_map = {}

            def load_x(bb, sl, fine=False):
                HW2 = (KO // 2) * 256
                xts = []
                for hf in range(2):
                    xt = xin.tile([128, HW2], BF16, tag="xh", name="xt", bufs=4)
                    row = (bb * S + sl * 256) // 256
                    if fine and hf == 0:
                        # quarter-granularity so the first matmuls start as
                        # soon as the first k-tiles land (startup only)
                        for qt in range(4):
                            nc.sync.dma_start(
                                xt[:, ds(qt * HW2 // 4, HW2 // 4)],
                                xh[row, :, ds(hf * HW2 + qt * HW2 // 4, HW2 // 4)],
                            )
                    else:
                        nc.sync.dma_start(xt, xh[row, :, ds(hf * HW2, HW2)])
                    xts.append(xt)
                xt_map[(bb, sl)] = xts

            # weights ride the scalar-engine HWDGE ring so the sync ring can
            # stream x tiles immediately (startup would otherwise head-block
            # ~35us behind 10MB of weights).
            w_all = wq_pool.tile([128, KO, NM * 128], BF16)
            nc.scalar.dma_start(w_all[:, 0:4, :], wqkvh[:, 0:4, :])
            nc.scalar.dma_start(w_all[:, 4:8, :], wqkvh[:, 4:8, :])
            for ch in range(1, NCH):
                nc.scalar.dma_start(
                    w_all[:, ds(ch * KH, KH), :], wqkvh[:, ds(ch * KH, KH), :]
                )
            load_x(0, 0, fine=True)
            cos_sb = consts.tile([128, S], BF16)
            nc.sync.dma_start(cos_sb, cosT[:])
            sin_sb = consts.tile([128, S], BF16)
            nc.sync.dma_start(sin_sb, sinT[:])
            ident = consts.tile([128, 128], BF16)
            nc.scalar.dma_start(ident, identT[:])
            ones = consts.tile([128, 128], BF16)
            nc.scalar.dma_start(ones, onesT[:])
            masks = consts.tile([128, 4, 512], BF16)
            nc.scalar.dma_start(masks, masksT[:])
            nbias = consts.tile([128, 1], F32)
            nc.gpsimd.memset(nbias, -4.0)
            wo = wq_pool.tile([128, NQ, H], BF16)
            nc.scalar.dma_start(wo, woh[:])

            def emit_oproj_block(b, tm):
                """o_proj for token tile tm of batch b: 8 x [128,512] out cols.

                Evacuation is all-DVE: scalar must stay free for attention
                EXPs these blocks are interleaved with.
                """
                attn_b = attn_tiles[b]
                ot = ot_pool.tile([128, H], F16, tag="ot", name="ot")
                for hs in range(8):
                    po = psum.tile([128, 512], F32, tag="sp", name="po", bufs=4)
                    for a in range(NQ):
                        nc.tensor.matmul(
                            po,
                            attn_b[:, a, ts(tm, 128)],
                            wo[:, a, ds(hs * 512, 512)],
                            start=(a == 0),
                            stop=(a == NQ - 1),
                        )
                    nc.vector.tensor_copy(ot[:, ds(hs * 512, 512)], po)
                    if hs == 3:
                        nc.sync.dma_start(
                            out[ds(b * S + tm * 128, 128), 0:2048], ot[:, 0:2048]
                        )
                nc.sync.dma_start(
                    out[ds(b * S + tm * 128, 128), 2048:4096], ot[:, 2048:4096]
                )

            def emit_attn_head(qkv_b, vn, attn_b, j, h, defer_tail=False):
                nkt = 4 * (j + 1)
                qs = qkv_b[:, h, ds(j * 512, 512)]
                pv = psum.tile([128, 512], F32, tag="pv", name="pv", bufs=1)
                acc = sm_pool.tile([128, 512], BF16, tag="acc", name="acc")
                pts = []
                for half in range(j + 1):
                    for kk in range(4):
                        kt = half * 4 + kk
                        sp = psum.tile([128, 512], F32, tag="sp", name="sp", bufs=4)
                        nc.tensor.matmul(
                            sp, qkv_b[:, NQ, ts(kt, 128)], qs, start=True, stop=True
                        )
                        pt = pt_pool.tile([128, 512], BF16, tag="pt", name="pt")
                        nc.scalar.activation(
                            pt, sp, EXPF, bias=nbias[:, 0:1], scale=SCALE
                        )
                        if kt >= j * 4:
                            nc.vector.tensor_mul(pt, pt, masks[:, kt - j * 4, :])
                        if kt == 0:
                            nc.vector.tensor_copy(acc, pt)
                        else:
                            nc.vector.tensor_add(acc, acc, pt)
                        pts.append(pt)
                    for kk in range(4):
                        kt = half * 4 + kk
                        nc.tensor.matmul(
                            pv,
                            vn[:, kt, :],
                            pts[kt],
                            start=(kt == 0),
                            stop=(kt == nkt - 1),
                        )

                def tail():
                    # denominator: all-ones matmul = partition-reduce+broadcast
                    rbm = psum.tile([128, 512], F32, tag="sp", name="rbm", bufs=4)
                    nc.tensor.matmul(rbm, ones, acc, start=True, stop=True)
                    recip = sm_pool.tile([128, 512], F32, tag="recip", name="recip")
                    nc.vector.reciprocal_approx_fast(recip, rbm)
                    nc.vector.tensor_mul(attn_b[:, h, ds(j * 512, 512)], pv, recip)

                if defer_tail:
                    return tail
                tail()
                return None

            for b in range(B):
                # ---------------- A(b): QKV projection + RoPE ----------------
                # 256-token slabs: the 6 fp32 accumulators fit 3 PSUM banks,
                # leaving 4 ("sp") + 1 ("pv") banks for the attention/o_proj
                # work this phase overlaps with.
                qkv_b = qkv_pool.tile([128, NM, S], BF16, tag="qkv", name="qkv_b")
                SLAB = 256
                XH = KO // 2  # x k-tiles per half-slab tile
                for sl in range(S // SLAB):
                    tok0 = b * S + sl * SLAB
                    tq = ds(sl * SLAB, SLAB)
                    # whole x slab resident; each m-group streams it again
                    if (b, sl) not in xt_map:
                        load_x(b, sl)
                    xts = xt_map.pop((b, sl))
                    if sl == 1 and pending_tail is not None:
                        # flush the previous batch's last softmax tail behind
                        # this batch's first slab of projection matmuls
                        pending_tail()
                        pending_tail = None
                    for g in range(2):
                        pss = [
                            psum.tile(
                                [128, SLAB], F32, tag="pa", name=f"qk_ps{mi}", bufs=3
                            )
                            for mi in range(3)
                        ]
                        for hf in range(2):
                            for k in range(XH):
                                for mi in range(3):
                                    nc.tensor.matmul(
                                        pss[mi],
                                        w_all[:, hf * XH + k, ts(g * 3 + mi, 128)],
                                        xts[hf][:, ds(k * SLAB, SLAB)],
                                        start=(hf == 0 and k == 0),
                                        stop=(hf == 1 and k == XH - 1),
                                    )
                        for mi in range(3):
                            nc.scalar.copy(qkv_b[:, g * 3 + mi, tq], pss[mi])
                    # RoPE on q heads + k head (rows 0..4), not v
                    rot = rot_pool.tile([128, NQ + 1, SLAB], BF16, tag="rot", name="rot")
                    nc.scalar.dma_start(rot[0:64], qkv_b[64:128, 0 : NQ + 1, tq])
                    nc.scalar.dma_start(rot[64:128], qkv_b[0:64, 0 : NQ + 1, tq])
                    s0 = sl * SLAB
                    cos_br = cos_sb[:, None, ds(s0, SLAB)].to_broadcast(
                        (128, NQ + 1, SLAB)
                    )
                    sin_br = sin_sb[:, None, ds(s0, SLAB)].to_broadcast(
                        (128, NQ + 1, SLAB)
                    )
                    nc.vector.tensor_mul(
                        qkv_b[:, 0 : NQ + 1, tq], qkv_b[:, 0 : NQ + 1, tq], cos_br
                    )
                    nc.vector.tensor_mul(rot, rot, sin_br)
                    nc.vector.tensor_add(
                        qkv_b[:, 0 : NQ + 1, tq], qkv_b[:, 0 : NQ + 1, tq], rot
                    )

                if b + 1 < B:
                    # prefetch next batch's first x slab: its DMA issue must
                    # not queue behind B(b)'s output-DMA issues on this ring
                    load_x(b + 1, 0)

                # V transposes (only need the V row of qkv_b — no RoPE dep)
                vn = vn_pool.tile([128, NKT, 128], BF16, tag="vn", name="vn")
                for kt in range(NKT):
                    tp = psum.tile([128, 128], BF16, tag="sp", name="tp", bufs=4)
                    nc.tensor.transpose(tp, qkv_b[:, NQ + 1, ts(kt, 128)], ident)
                    nc.vector.tensor_copy(vn[:, kt, :], tp)

                attn_b = attn_pool.tile([128, NQ, S], BF16, tag="attn", name="attn_b")
                attn_tiles[b] = attn_b

                # ---- B(b) attention, with C(b-1) o_proj blocks interleaved
                fill = list(range(8)) if b > 0 else []
                if fill:  # cover the RoPE tail of A(b) before the first head
                    emit_oproj_block(b - 1, fill.pop(0))
                    emit_oproj_block(b - 1, fill.pop(0))
                for j in range(2):
                    for h in range(NQ):
                        defer = j == 1 and h == NQ - 1
                        t = emit_attn_head(qkv_b, vn, attn_b, j, h, defer_tail=defer)
                        if defer:
                            pending_tail = t
                        # keep one fill block for after the last head so its
                        # EXP/PV chain is covered before the next batch starts
                        last_head = j == 1 and h == NQ - 1
                        if fill and (len(fill) > 1 or last_head):
                            emit_oproj_block(b - 1, fill.pop(0))

            # final batch: the deferred tail only writes the j=1 half of the
            # last head, which o_proj token tiles 4-7 read — flush it behind
            # the first four (j=0) tiles
            for tm in range(4):
                emit_oproj_block(B - 1, tm)
            if pending_tail is not None:
                pending_tail()
                pending_tail = None
            for tm in range(4, 8):
                emit_oproj_block(B - 1, tm)
    nc.compile()
    return nc


def prep_core_inputs(hidden_bf, w_qkv, w_o, core: int):
    """Per-core input map; all matmul-path tensors pre-laid-out on host."""
    bf = ml_dtypes.bfloat16
    NQ_TOT = N_CORES * NQ
    q0 = core * NQ * D
    rows = list(range(q0, q0 + NQ * D))
    rows += list(range(NQ_TOT * D + core * D, NQ_TOT * D + (core + 1) * D))
    rows += list(
        range((NQ_TOT + N_CORES) * D + core * D, (NQ_TOT + N_CORES) * D + (core + 1) * D)
    )
    wqkvh = np.ascontiguousarray(
        w_qkv[rows, :].T.astype(bf).reshape(KO, 128, NM * 128).transpose(1, 0, 2)
    )
    woh = np.ascontiguousarray(
        w_o[:, core * NQ * D : (core + 1) * NQ * D]
        .T.astype(bf)
        .reshape(NQ, 128, H)
        .transpose(1, 0, 2)
    )
    return {"xh": hidden_bf, "wqkvh": wqkvh, "woh": woh}


def prep_shared_inputs(rope_theta=10000.0):
    bf = ml_dtypes.bfloat16
    inv = 1.0 / (rope_theta ** (np.arange(0, D, 2, dtype=np.float64) / D))
    ang = np.arange(S, dtype=np.float64)[:, None] * inv[None, :]  # [S, 64]
    cos = np.cos(ang).T.astype(np.float32)  # [64, S]
    sin = np.sin(ang).T.astype(np.float32)
    cosT = np.concatenate([cos, cos], axis=0)  # [128, S]
    sinT = np.concatenate([-sin, sin], axis=0)
    kl = np.arange(128)[:, None, None]
    r = np.arange(4)[None, :, None]
    ql = np.arange(512)[None, None, :]
    masksT = ((r * 128 + kl) <= ql).astype(np.float32)
    return {
        "cosT": np.ascontiguousarray(cosT.astype(bf)),
        "sinT": np.ascontiguousarray(sinT.astype(bf)),
        "masksT": np.ascontiguousarray(masksT.astype(bf)),
        "identT": np.eye(128, dtype=bf),
        "onesT": np.ones((128, 128), dtype=bf),
    }


_CACHE = {}
LAST_EXEC_NS = None


def kernel(hidden_states=None, w_qkv=None, w_o=None, seq_len=None, **_):
    bf = ml_dtypes.bfloat16
    hidden = np.asarray(hidden_states, dtype=np.float32)
    w_qkv = np.asarray(w_qkv, dtype=np.float32)
    w_o = np.asarray(w_o, dtype=np.float32)

    if "nc" not in _CACHE:
        _CACHE["nc"] = build_nc()
    nc = _CACHE["nc"]

    # x^T laid out [128, KO, T] once, shared across cores
    hidden_bf = np.ascontiguousarray(
        hidden.T.astype(bf)
        .reshape(KO, 128, T // 256, 256)
        .transpose(2, 1, 0, 3)
        .reshape(T // 256, 128, KO * 256)
    )
    shared = prep_shared_inputs()
    in_maps = []
    for c in range(N_CORES):
        m = dict(shared)
        m.update(prep_core_inputs(hidden_bf, w_qkv, w_o, c))
        in_maps.append(m)

    trace = os.environ.get("KERNEL_TRACE", "0") == "1"
    res = run_bass_kernel_spmd(
        nc, in_maps, core_ids=list(range(N_CORES)), trace=trace
    )
    global LAST_EXEC_NS
    if res.exec_time_ns is not None:
        LAST_EXEC_NS = res.exec_time_ns
        print(f"HW exec time: {res.exec_time_ns} ns")
        if res.instructions_and_trace is not None:
            print(f"trace: {res.instructions_and_trace[1]}")
    outs = [r["o_partial"] for r in res.results]
    acc = np.zeros(outs[0].shape, dtype=np.float32)
    for o in outs:
        acc += o.astype(np.float32)
    return acc
